# revision 1
# baseline (speedup 1.0000x reference)
"""CQAttention Trainium2 kernel — data-parallel over batch across 8 NeuronCores.

Problem shapes (hardcoded): B=32, H=256, Lc=1024, Lq=256.
Each core processes B/8 = 4 batches.

Math (per batch, with all-ones masks — guaranteed by the problem spec):
  Ct = C^T [Lc,H], Qt = Q^T [Lq,H]
  S[l,m] = Ct[l]@w1 + Qt[m]@w2 + (Ct[l]*w3)@Qt[m]
  Z = exp(S + r[l] + q[m]) serves BOTH softmaxes:
    S_row = Z / rowsum(Z)   (row term r cancels in row softmax)
    S_col = Z / colsum(Z)   (col term q cancels in col softmax)
  A  = S_row @ Qt
  Bv = S_row @ (S_col^T @ Ct)      (factored: avoids the Lc x Lc product)
  out = relu([Ct, A, Ct*A, Ct*Bv] @ W_res^T + b_res)^T  -> [H, Lc]

Implementation notes:
  - The S (logit) matmuls run in fp32r: full PE rate (1 cycle/row for
    N>=256) with near-fp32 accuracy; plain fp32 matmuls are 4x slower.
  - Everything downstream of exp (attention weights in [0,1], T, A, Bv,
    final projection) runs in bf16 (rel err ~3e-3 total, gate is 2e-2).
  - Transposes (W^T once, C^T/Q^T per batch) are PE transposes of the
    bf16 copies (1 cycle/row). DMA-xbar transposes were tried and are
    both hazardous (xbar output corrupts when DmaTranspose shares a
    HWDGE queue with DMACopy — this Tile version does not serialize
    them) and slower end-to-end on every queue arrangement tested.
  - exp's accum_out produces the row/col softmax sums for free; the bias
    terms r[l], q[m] are folded into the stationary matmul operands
    (CA = C*w3+w2, QA = Q*w3+w1) and the exp's per-partition bias
    (r_col/q_col via a small DRAM-bounce layout shuffle).
  - Emission is software-pipelined: frontend(b+1) (DMA loads, bf16
    casts, transposes, CA/QA, r/q) is emitted before backend(b), which
    removed ~60us of PE idle at batch boundaries.
"""

import numpy as np

_CACHE = {}

B_FULL = 32
N_CORES = 8
BB = B_FULL // N_CORES  # batches per core = 4
H = 256
LC = 1024
LQ = 256


def _build(reps: int = 1):
    from contextlib import ExitStack

    import concourse.bass as bass
    import concourse.tile as tile
    from concourse import bacc, mybir
    from concourse.masks import make_identity

    f32 = mybir.dt.float32
    f32r = mybir.dt.float32r
    bf16 = mybir.dt.bfloat16
    AF = mybir.ActivationFunctionType
    OP = mybir.AluOpType

    nc = bacc.Bacc("TRN2", target_bir_lowering=False, debug=False)

    def mm(out, lhsT, rhs, start, stop):
        # fp32r runs the PE at full rate (1 cycle/row for N>=256) vs 4x for fp32
        nc.tensor.matmul(
            out,
            lhsT=lhsT.bitcast(f32r),
            rhs=rhs.bitcast(f32r),
            start=start,
            stop=stop,
        )

    def mmb(out, lhsT, rhs, start, stop):
        nc.tensor.matmul(out, lhsT=lhsT, rhs=rhs, start=start, stop=stop)

    C = nc.dram_tensor("C", [BB, H, LC], f32, kind="ExternalInput")
    Q = nc.dram_tensor("Q", [BB, H, LQ], f32, kind="ExternalInput")
    w = nc.dram_tensor("w", [3 * H], f32, kind="ExternalInput")
    W_res = nc.dram_tensor("W_res", [H, 4 * H], f32, kind="ExternalInput")
    b_res = nc.dram_tensor("b_res", [H], f32, kind="ExternalInput")
    out = nc.dram_tensor("out", [BB, H, LC], f32, kind="ExternalOutput")

    KH = H // 128  # 2 h-chunks
    NLT = LC // 128  # 8 l-tiles
    NMT = LQ // 128  # 2 m-tiles

    with tile.TileContext(nc) as tc:
        with ExitStack() as ctx:
            singles = ctx.enter_context(tc.tile_pool(name="singles", bufs=1))
            sb = ctx.enter_context(tc.tile_pool(name="sb", bufs=2))
            sb1 = ctx.enter_context(tc.tile_pool(name="sb1", bufs=3))
            sbig = ctx.enter_context(tc.tile_pool(name="sbig", bufs=2))
            sbig1 = ctx.enter_context(tc.tile_pool(name="sbig1", bufs=3))
            ps_tr = ctx.enter_context(
                tc.tile_pool(name="ps_tr", bufs=2, space="PSUM")
            )
            ps_z = ctx.enter_context(
                tc.tile_pool(name="ps_z", bufs=2, space="PSUM")
            )
            ps_big = ctx.enter_context(
                tc.tile_pool(name="ps_big", bufs=2, space="PSUM")
            )
            dr = ctx.enter_context(tc.tile_pool(name="dr", bufs=2, space="DRAM"))

            # ---- one-time constants ----
            identity_bf = singles.tile([128, 128], bf16)
            make_identity(nc, identity_bf)

            w1_col = singles.tile([128, KH], f32r)
            w2_col = singles.tile([128, KH], f32r)
            w3_col = singles.tile([128, KH], f32)
            nc.sync.dma_start(
                out=w1_col,
                in_=w.ap()[0:H].rearrange("(i p) -> p i", i=KH, p=128).bitcast(f32r),
            )
            nc.sync.dma_start(
                out=w2_col,
                in_=w.ap()[H : 2 * H]
                .rearrange("(i p) -> p i", i=KH, p=128)
                .bitcast(f32r),
            )
            nc.sync.dma_start(
                out=w3_col,
                in_=w.ap()[2 * H : 3 * H].rearrange("(i p) -> p i", i=KH, p=128),
            )
            b_col = singles.tile([128, KH], f32)
            nc.sync.dma_start(
                out=b_col, in_=b_res.ap().rearrange("(i p) -> p i", i=KH, p=128)
            )

            # W_res^T (bf16): WT[f][p, ho] = W_res[ho, 128*f + p]
            WT = []
            for f in range(8):
                t_wt = singles.tile([128, H], bf16, tag=f"wt{f}")
                WT.append(t_wt)
            for j in range(KH):
                t = singles.tile([128, 4 * H], f32, tag=f"wn{j}")
                nc.sync.dma_start(out=t, in_=W_res.ap()[128 * j : 128 * (j + 1), :])
                tb = singles.tile([128, 4 * H], bf16, tag=f"wnb{j}")
                nc.vector.tensor_copy(tb, t)
                for f in range(8):
                    pt = ps_tr.tile([128, 128], bf16, tag="tr")
                    nc.tensor.transpose(
                        pt, tb[:, 128 * f : 128 * (f + 1)], identity_bf
                    )
                    nc.any.tensor_copy(
                        out=WT[f][:, 128 * j : 128 * (j + 1)], in_=pt
                    )

            def frontend(b):
                st = {}
                # ---- load ----
                C_nat = []
                Q_nat = []
                for k in range(KH):
                    t = sbig.tile([128, LC], f32r, tag=f"cnat{k}")
                    nc.sync.dma_start(
                        out=t,
                        in_=C.ap()[b, 128 * k : 128 * (k + 1), :].bitcast(f32r),
                    )
                    C_nat.append(t)
                    tq = sb.tile([128, LQ], f32r, tag=f"qnat{k}")
                    nc.sync.dma_start(
                        out=tq,
                        in_=Q.ap()[b, 128 * k : 128 * (k + 1), :].bitcast(f32r),
                    )
                    Q_nat.append(tq)

                # ---- bf16 copies + DMA-xbar transposes ----
                C_bf = []
                Q_bf = []
                for k in range(KH):
                    cb = sbig.tile([128, LC], bf16, tag=f"cbf{k}")
                    nc.vector.tensor_copy(cb, C_nat[k].bitcast(f32))
                    C_bf.append(cb)
                    qb = sb.tile([128, LQ], bf16, tag=f"qbf{k}")
                    nc.vector.tensor_copy(qb, Q_nat[k].bitcast(f32))
                    Q_bf.append(qb)

                # CtT[i][p, h] = C^T[128*i + p, h];  QT[j][p, h] = Q^T[128*j + p, h]
                CtT = []
                for i in range(NLT):
                    t_ct = sb1.tile([128, H], bf16, tag=f"ctt{i}")
                    for k in range(KH):
                        pt = ps_tr.tile([128, 128], bf16, tag="tr")
                        nc.tensor.transpose(
                            pt, C_bf[k][:, 128 * i : 128 * (i + 1)], identity_bf
                        )
                        nc.any.tensor_copy(
                            out=t_ct[:, 128 * k : 128 * (k + 1)], in_=pt
                        )
                    CtT.append(t_ct)
                QT = []
                for j in range(NMT):
                    t_qt = sb1.tile([128, H], bf16, tag=f"qt{j}")
                    for k in range(KH):
                        pt = ps_tr.tile([128, 128], bf16, tag="tr")
                        nc.tensor.transpose(
                            pt, Q_bf[k][:, 128 * j : 128 * (j + 1)], identity_bf
                        )
                        nc.any.tensor_copy(
                            out=t_qt[:, 128 * k : 128 * (k + 1)], in_=pt
                        )
                    QT.append(t_qt)

                # ---- affine-augmented operands ----
                CA = []
                QA = []
                for k in range(KH):
                    t = sbig.tile([128, LC], f32r, tag=f"ca{k}")
                    nc.vector.tensor_scalar(
                        out=t,
                        in0=C_nat[k],
                        scalar1=w3_col[:, k : k + 1],
                        scalar2=w2_col[:, k : k + 1].bitcast(f32),
                        op0=OP.mult,
                        op1=OP.add,
                    )
                    CA.append(t)
                    tq = sb.tile([128, LQ], f32r, tag=f"qa{k}")
                    nc.vector.tensor_scalar(
                        out=tq,
                        in0=Q_nat[k],
                        scalar1=w3_col[:, k : k + 1],
                        scalar2=w1_col[:, k : k + 1].bitcast(f32),
                        op0=OP.mult,
                        op1=OP.add,
                    )
                    QA.append(tq)

                # ---- r,q bias rows -> per-partition columns (DRAM bounce) ----
                r_row = sb.tile([1, LC], f32, tag="rrow")
                for c in range(2):
                    ps_r = ps_z.tile([1, 512], f32, tag="z")
                    for k in range(KH):
                        mm(
                            ps_r,
                            w1_col[:, k : k + 1],
                            C_nat[k][:, 512 * c : 512 * (c + 1)],
                            (k == 0),
                            (k == KH - 1),
                        )
                    nc.any.tensor_copy(
                        out=r_row[:, 512 * c : 512 * (c + 1)], in_=ps_r
                    )
                r_dram = dr.tile([1, LC], f32, tag="rd")
                nc.sync.dma_start(out=r_dram, in_=r_row)
                r_col = sb.tile([128, NLT], f32, tag="rcol")
                nc.sync.dma_start(
                    out=r_col,
                    in_=r_dram.rearrange("1 (i p) -> p i", i=NLT, p=128),
                )

                ps_q = ps_z.tile([1, LQ], f32, tag="z")
                for k in range(KH):
                    mm(
                        ps_q,
                        w2_col[:, k : k + 1],
                        Q_nat[k],
                        (k == 0),
                        (k == KH - 1),
                    )
                q_row = sb.tile([1, LQ], f32, tag="qrow")
                nc.any.tensor_copy(out=q_row, in_=ps_q)
                q_dram = dr.tile([1, LQ], f32, tag="qd")
                nc.sync.dma_start(out=q_dram, in_=q_row)
                q_col = sb.tile([128, NMT], f32, tag="qcol")
                nc.sync.dma_start(
                    out=q_col,
                    in_=q_dram.rearrange("1 (i p) -> p i", i=NMT, p=128),
                )

                st.update(
                    C_nat=C_nat, Q_nat=Q_nat, C_bf=C_bf, CtT=CtT, QT=QT,
                    CA=CA, QA=QA, r_col=r_col, q_col=q_col,
                )
                return st

            def backend(b, st):
                C_nat = st["C_nat"]; Q_nat = st["Q_nat"]; C_bf = st["C_bf"]
                CtT = st["CtT"]; QT = st["QT"]; CA = st["CA"]; QA = st["QA"]
                r_col = st["r_col"]; q_col = st["q_col"]

                if True:
                    # ---- Z in [l, m] layout + rowsums rho ----
                    rho_col = sb.tile([128, NLT], f32, tag="rho")
                    E_lm = []
                    for i in range(NLT):
                        pz = ps_z.tile([128, LQ], f32, tag="z")
                        for k in range(KH):
                            mm(
                                pz,
                                CA[k][:, 128 * i : 128 * (i + 1)],
                                Q_nat[k],
                                (k == 0),
                                (k == KH - 1),
                            )
                        e = sb1.tile([128, LQ], bf16, tag=f"elm{i}")
                        nc.scalar.activation(
                            out=e,
                            in_=pz,
                            func=AF.Exp,
                            bias=r_col[:, i : i + 1],
                            accum_out=rho_col[:, i : i + 1],
                        )
                        E_lm.append(e)

                    # ---- Z in [m, l] layout + colsums kappa ----
                    kap_col = sb.tile([128, NMT], f32, tag="kap")
                    E_ml = []
                    for j in range(NMT):
                        pzt = ps_big.tile([128, LC], f32, tag="big")
                        for k in range(KH):
                            for c in range(2):
                                mm(
                                    pzt[:, 512 * c : 512 * (c + 1)],
                                    QA[k][:, 128 * j : 128 * (j + 1)],
                                    C_nat[k][:, 512 * c : 512 * (c + 1)],
                                    (k == 0),
                                    (k == KH - 1),
                                )
                        e = sbig1.tile([128, LC], bf16, tag=f"eml{j}")
                        nc.scalar.activation(
                            out=e,
                            in_=pzt,
                            func=AF.Exp,
                            bias=q_col[:, j : j + 1],
                            accum_out=kap_col[:, j : j + 1],
                        )
                        E_ml.append(e)

                    # ---- reciprocals ----
                    rho_inv = sb.tile([128, NLT], f32, tag="rhoi")
                    nc.vector.reciprocal(rho_inv, rho_col)
                    kap_inv = sb.tile([128, NMT], f32, tag="kapi")
                    nc.vector.reciprocal(kap_inv, kap_col)

                    # rho_inv -> bf16 row layout, broadcast to all partitions
                    rho_inv_bf = sb.tile([128, NLT], bf16, tag="rhoib")
                    nc.vector.tensor_copy(rho_inv_bf, rho_inv)
                    ri_dram = dr.tile([1, LC], bf16, tag="rid")
                    nc.sync.dma_start(
                        out=ri_dram.rearrange("1 (i p) -> p i", i=NLT, p=128),
                        in_=rho_inv_bf,
                    )
                    ri_bc = sbig1.tile([128, LC], bf16, tag="ribc")
                    bc_src = bass.AP(
                        tensor=ri_dram.tensor,
                        offset=ri_dram.offset,
                        ap=[[0, 128], [1, LC]],
                    )
                    nc.sync.dma_start(out=ri_bc, in_=bc_src)

                    # ---- P^T = Z^T / rho  (row-softmax, transposed layout) ----
                    P_ml = []
                    for j in range(NMT):
                        t = sbig1.tile([128, LC], bf16, tag=f"pml{j}")
                        nc.vector.tensor_mul(t, E_ml[j], ri_bc)
                        P_ml.append(t)

                    # ---- T = S_col^T @ Ct   [m, h] ----
                    T_nat = []
                    for j in range(NMT):
                        pT = ps_z.tile([128, H], f32, tag="z")
                        for i in range(NLT):
                            mmb(
                                pT,
                                E_lm[i][:, 128 * j : 128 * (j + 1)],
                                CtT[i],
                                (i == 0),
                                (i == NLT - 1),
                            )
                        t = sb1.tile([128, H], bf16, tag=f"tn{j}")
                        nc.vector.tensor_scalar_mul(t, pT, kap_inv[:, j : j + 1])
                        T_nat.append(t)

                    # ---- A^T and Bv^T  [h, l] ----
                    A_T = []
                    Bv_T = []
                    for t_i in range(KH):
                        pA = ps_big.tile([128, LC], f32, tag="big")
                        for k in range(NMT):
                            for c in range(2):
                                mmb(
                                    pA[:, 512 * c : 512 * (c + 1)],
                                    QT[k][:, 128 * t_i : 128 * (t_i + 1)],
                                    P_ml[k][:, 512 * c : 512 * (c + 1)],
                                    (k == 0),
                                    (k == NMT - 1),
                                )
                        a = sbig1.tile([128, LC], bf16, tag=f"at{t_i}")
                        nc.any.tensor_copy(out=a, in_=pA)
                        A_T.append(a)
                    for t_i in range(KH):
                        pB = ps_big.tile([128, LC], f32, tag="big")
                        for k in range(NMT):
                            for c in range(2):
                                mmb(
                                    pB[:, 512 * c : 512 * (c + 1)],
                                    T_nat[k][:, 128 * t_i : 128 * (t_i + 1)],
                                    P_ml[k][:, 512 * c : 512 * (c + 1)],
                                    (k == 0),
                                    (k == NMT - 1),
                                )
                        bv = sbig1.tile([128, LC], bf16, tag=f"bvt{t_i}")
                        nc.any.tensor_copy(out=bv, in_=pB)
                        Bv_T.append(bv)

                    # ---- products ----
                    CA1 = []
                    CB1 = []
                    for t_i in range(KH):
                        p1 = sbig1.tile([128, LC], bf16, tag=f"ca1{t_i}")
                        nc.vector.tensor_mul(p1, C_bf[t_i], A_T[t_i])
                        CA1.append(p1)
                        p2 = sbig1.tile([128, LC], bf16, tag=f"cb1{t_i}")
                        nc.vector.tensor_mul(p2, C_bf[t_i], Bv_T[t_i])
                        CB1.append(p2)

                    # ---- final matmul + relu + store ----
                    blocks = [
                        C_bf[0],
                        C_bf[1],
                        A_T[0],
                        A_T[1],
                        CA1[0],
                        CA1[1],
                        CB1[0],
                        CB1[1],
                    ]
                    for t_i in range(KH):
                        po = ps_big.tile([128, LC], f32, tag="big")
                        for f in range(8):
                            for c in range(2):
                                mmb(
                                    po[:, 512 * c : 512 * (c + 1)],
                                    WT[f][:, 128 * t_i : 128 * (t_i + 1)],
                                    blocks[f][:, 512 * c : 512 * (c + 1)],
                                    (f == 0),
                                    (f == 7),
                                )
                        o = sbig.tile([128, LC], f32, tag=f"osb{t_i}")
                        nc.scalar.activation(
                            out=o,
                            in_=po,
                            func=AF.Relu,
                            bias=b_col[:, t_i : t_i + 1],
                        )
                        nc.sync.dma_start(
                            out=out.ap()[b, 128 * t_i : 128 * (t_i + 1), :], in_=o
                        )

            def body(iv=None):
                st_prev = None
                for b in range(BB):
                    st = frontend(b)
                    if st_prev is not None:
                        backend(b - 1, st_prev)
                    st_prev = st
                backend(BB - 1, st_prev)

            if reps == 1:
                body()
            else:
                with tc.For_i(0, reps, 1) as iv:
                    body(iv)

    nc.compile()
    return nc


def _get_nc(reps: int = 1):
    key = ("nc", reps)
    if key not in _CACHE:
        _CACHE[key] = _build(reps)
    return _CACHE[key]


def kernel(C, Q, cmask, qmask, w, W_res, b_res, _reps: int = 1, _want_res: bool = False):
    from concourse.bass_utils import run_bass_kernel_spmd

    nc = _get_nc(_reps)

    C = np.ascontiguousarray(C, dtype=np.float32)
    Q = np.ascontiguousarray(Q, dtype=np.float32)
    w = np.ascontiguousarray(w, dtype=np.float32)
    W_res = np.ascontiguousarray(W_res, dtype=np.float32)
    b_res = np.ascontiguousarray(b_res, dtype=np.float32)

    in_maps = []
    for i in range(N_CORES):
        sl = slice(i * BB, (i + 1) * BB)
        in_maps.append(
            {"C": C[sl], "Q": Q[sl], "w": w, "W_res": W_res, "b_res": b_res}
        )

    res = run_bass_kernel_spmd(nc, in_maps, core_ids=list(range(N_CORES)))
    out = np.concatenate([res.results[i]["out"] for i in range(N_CORES)], axis=0)
    if _want_res:
        return out, res
    return out



# revision 5
# speedup vs baseline: 2.5394x; 2.5394x over previous
"""CQAttention Trainium2 kernel — data-parallel over batch across 8 NeuronCores.

Problem shapes (hardcoded): B=32, H=256, Lc=1024, Lq=256.
Each core processes B/8 = 4 batches.

Math (per batch, all-ones masks guaranteed by the problem spec):
  Ct = C^T [Lc,H], Qt = Q^T [Lq,H]
  S[l,m] = Ct[l]@w1 + Qt[m]@w2 + (Ct[l]*w3)@Qt[m]
  Z = exp(S) serves BOTH softmaxes:
    S_row = Z / rowsum(Z),  S_col = Z / colsum(Z)
  A  = S_row @ Qt
  Bv = S_row @ (S_col^T @ Ct)      (factored: avoids the Lc x Lc product)
  out = relu([Ct, A, Ct*A, Ct*Bv] @ W_res^T + b_res)^T  -> [H, Lc]

Key structure (v2 — engine-load-balanced single-Z design):
  - Z is computed ONCE in [l,m] layout: pz = CA^T @ Q (+ q folded into
    CA = C*w3 + w2 along the free dim) + r via a K=1 matmul (lhsT = r_row
    slice, rhs = ones row) accumulated into the same PSUM — no DRAM
    bounce, no exp bias needed.
  - exp(pz) -> E_lm bf16 with accum_out giving rowsums rho for free.
  - Row-softmax weights P = E_lm * rho_inv (per-partition scalar on
    GpSimd), then PE-transposed into P_ml [m,l] (replaces the second
    Z^T matmul + exp of the baseline).
  - kappa (colsums of Z) via N=1 ones-matmuls sharing the E_lm lhsT of
    the T matmuls -> lands directly in [m-partition] column layout.
  - All PE transposes pack into wide PSUM tiles -> one DVE copy per
    1024 columns instead of per-128 copies.
  - 1-input elementwise (casts, CA, P scaling) runs on the otherwise
    idle GpSimd engine; DVE keeps PSUM reads and 2-input muls; ACT does
    only exp and the final bias+relu.
  - Output is stored bf16 (halves store traffic); converted to f32 on host.
"""

import numpy as np

_CACHE = {}

B_FULL = 32
N_CORES = 8
BB = B_FULL // N_CORES  # batches per core = 4
H = 256
LC = 1024
LQ = 256


def _build(reps: int = 1):
    from contextlib import ExitStack

    import concourse.bass as bass
    import concourse.tile as tile
    from concourse import bacc, mybir
    from concourse.masks import make_identity

    f32 = mybir.dt.float32
    f32r = mybir.dt.float32r
    bf16 = mybir.dt.bfloat16
    AF = mybir.ActivationFunctionType
    OP = mybir.AluOpType

    nc = bacc.Bacc("TRN2", target_bir_lowering=False, debug=False)

    def mm(out, lhsT, rhs, start, stop):
        # fp32r runs the PE at full rate (1 cycle/row for N>=256)
        nc.tensor.matmul(
            out,
            lhsT=lhsT.bitcast(f32r),
            rhs=rhs.bitcast(f32r),
            start=start,
            stop=stop,
        )

    def mmb(out, lhsT, rhs, start, stop):
        nc.tensor.matmul(out, lhsT=lhsT, rhs=rhs, start=start, stop=stop)

    C = nc.dram_tensor("C", [BB, H, LC], f32, kind="ExternalInput")
    Q = nc.dram_tensor("Q", [BB, H, LQ], f32, kind="ExternalInput")
    w = nc.dram_tensor("w", [3 * H], f32, kind="ExternalInput")
    W_res = nc.dram_tensor("W_res", [H, 4 * H], f32, kind="ExternalInput")
    b_res = nc.dram_tensor("b_res", [H], f32, kind="ExternalInput")
    out = nc.dram_tensor("out", [BB, H, LC], bf16, kind="ExternalOutput")

    KH = H // 128  # 2 h-chunks
    NLT = LC // 128  # 8 l-tiles
    NMT = LQ // 128  # 2 m-tiles

    with tile.TileContext(nc) as tc:
        with ExitStack() as ctx:
            singles = ctx.enter_context(tc.tile_pool(name="singles", bufs=1))
            sb = ctx.enter_context(tc.tile_pool(name="sb", bufs=2))
            sb3 = ctx.enter_context(tc.tile_pool(name="sb3", bufs=3))
            sbL = ctx.enter_context(tc.tile_pool(name="sbL", bufs=BB))
            ps_m = ctx.enter_context(
                tc.tile_pool(name="ps_m", bufs=2, space="PSUM")
            )
            ps_g = ctx.enter_context(
                tc.tile_pool(name="ps_g", bufs=4, space="PSUM")
            )

            # ---- one-time constants ----
            identity_bf = singles.tile([128, 128], bf16)
            make_identity(nc, identity_bf)

            # w1 feeds only the tiny N=1 r-matmuls, which run in plain fp32
            # (fp32r has ISA restrictions at small free dims)
            w1_col = singles.tile([128, KH], f32)
            nc.sync.dma_start(
                out=w1_col,
                in_=w.ap()[0:H].rearrange("(i p) -> p i", i=KH, p=128),
            )
            w23 = singles.tile([128, 2 * KH], f32)
            nc.sync.dma_start(
                out=w23,
                in_=w.ap()[H : 3 * H].rearrange("(i p) -> p i", i=2 * KH, p=128),
            )
            w2_col = w23[:, 0:KH]
            w3_col = w23[:, KH : 2 * KH]
            b_col = singles.tile([128, KH], f32)
            nc.sync.dma_start(
                out=b_col, in_=b_res.ap().rearrange("(i p) -> p i", i=KH, p=128)
            )

            ones_col = singles.tile([128, 1], bf16)
            nc.vector.memset(ones_col, 1.0)

            WT_sb = []
            _Wn = []

            def load_W():
                # W_res loads + bf16 casts (issued early; transposes come later)
                for j in range(KH):
                    t = singles.tile([128, 4 * H], f32, tag=f"wn{j}")
                    nc.sync.dma_start(
                        out=t, in_=W_res.ap()[128 * j : 128 * (j + 1), :]
                    )
                    tb = singles.tile([128, 4 * H], bf16, tag=f"wnb{j}")
                    nc.gpsimd.tensor_copy(tb, t)
                    _Wn.append(tb)

            def build_WT():
                # W_res^T (bf16), packed: WT_sb[g][:, 256*fl + 128*j] holds
                # W_res[128j:128j+128, 128*(4g+fl):...]^T — own PSUM tag so the
                # one-time build never blocks the per-batch "m" slot rotation.
                for g in range(2):
                    # [128,1024] bf16 = same bytes as the [128,512] f32 "g"
                    # slots, so this borrows the ps_g pool without growing it.
                    pw = ps_g.tile([128, 1024], bf16, tag="g")
                    for fl in range(4):
                        f = 4 * g + fl
                        for j in range(KH):
                            nc.tensor.transpose(
                                pw[:, 256 * fl + 128 * j : 256 * fl + 128 * (j + 1)],
                                _Wn[j][:, 128 * f : 128 * (f + 1)],
                                identity_bf,
                            )
                    t_wt = singles.tile([128, 1024], bf16, tag=f"wt{g}")
                    nc.vector.tensor_copy(t_wt, pw)
                    WT_sb.append(t_wt)

            def WT(f):  # [128 (f-part), H]
                return WT_sb[f // 4][:, 256 * (f % 4) : 256 * (f % 4 + 1)]

            NCH = 1  # C column chunks (finer DMA granularity for the ramp)
            CW = LC // NCH
            TPC = NLT // NCH  # l-tiles per chunk

            def load(b):
                # C loads in NCH column chunks per k so downstream per-chunk
                # casts/CA/E-matmuls start as soon as the first chunk lands.
                C_nat = []
                Q_nat = []
                for k in range(KH):
                    chunks = []
                    for h in range(NCH):
                        t = sbL.tile([128, CW], f32r, tag=f"cnat{k}_{h}")
                        nc.sync.dma_start(
                            out=t,
                            in_=C.ap()[
                                b, 128 * k : 128 * (k + 1), CW * h : CW * (h + 1)
                            ].bitcast(f32r),
                        )
                        chunks.append(t)
                    C_nat.append(chunks)
                    tq = sbL.tile([128, LQ], f32r, tag=f"qnat{k}")
                    nc.sync.dma_start(
                        out=tq,
                        in_=Q.ap()[b, 128 * k : 128 * (k + 1), :].bitcast(f32r),
                    )
                    Q_nat.append(tq)
                return {"C_nat": C_nat, "Q_nat": Q_nat}

            def frontend(b, st):
                C_nat = st["C_nat"]
                Q_nat = st["Q_nat"]
                # ---- bf16 copies + CA = C*w3 + w2 (GpSimd, per chunk) ----
                C_bf = []
                Q_bf = []
                CA = []
                for k in range(KH):
                    cb = sb.tile([128, LC], bf16, tag=f"cbf{k}")
                    ca_ch = []
                    for h in range(NCH):
                        t = sb.tile([128, CW], f32r, tag=f"ca{k}_{h}")
                        nc.gpsimd.tensor_scalar(
                            out=t,
                            in0=C_nat[k][h].bitcast(f32),
                            scalar1=w3_col[:, k : k + 1],
                            scalar2=w2_col[:, k : k + 1],
                            op0=OP.mult,
                            op1=OP.add,
                        )
                        ca_ch.append(t)
                        nc.gpsimd.tensor_copy(
                            cb[:, CW * h : CW * (h + 1)], C_nat[k][h].bitcast(f32)
                        )
                    CA.append(ca_ch)
                    C_bf.append(cb)
                    qb = sb.tile([128, LQ], bf16, tag=f"qbf{k}")
                    nc.gpsimd.tensor_copy(qb, Q_nat[k].bitcast(f32))
                    Q_bf.append(qb)

                def CAs(k, i):  # [128, 128] slice of CA for l-tile i
                    return CA[k][i // TPC][:, 128 * (i % TPC) : 128 * (i % TPC + 1)]

                # ---- r_col[p, i] = sum_h C[h, 128i+p] w1[h] (N=1 matmuls):
                #      lands directly in the per-partition layout exp's bias wants.
                r_ps = ps_m.tile([128, NLT], f32, tag="m")
                for i in range(NLT):
                    for k in range(KH):
                        nc.tensor.matmul(
                            r_ps[:, i : i + 1],
                            lhsT=C_nat[k][i // TPC][
                                :, 128 * (i % TPC) : 128 * (i % TPC + 1)
                            ].bitcast(f32),
                            rhs=w1_col[:, k : k + 1],
                            start=(k == 0),
                            stop=(k == KH - 1),
                        )
                r_col = sb.tile([128, NLT], f32, tag="rcol")
                nc.vector.tensor_copy(r_col, r_ps)

                # ---- C^T packed: CtT_sb[p][:, 256*q + 128*k] = (C_bf[k][:, 128i:...])^T
                #      for i = 4p + q ;  CtT(i) = CtT_sb[i//4][:, 256*(i%4):+256]
                CtT_sb = []
                for p in range(2):
                    pt = ps_m.tile([128, 1024], bf16, tag="m")
                    for q in range(4):
                        i = 4 * p + q
                        for k in range(KH):
                            nc.tensor.transpose(
                                pt[:, 256 * q + 128 * k : 256 * q + 128 * (k + 1)],
                                C_bf[k][:, 128 * i : 128 * (i + 1)],
                                identity_bf,
                            )
                    t_ct = sb3.tile([128, 1024], bf16, tag=f"ctt{p}")
                    nc.vector.tensor_copy(t_ct, pt)
                    CtT_sb.append(t_ct)

                def CtT(i):
                    return CtT_sb[i // 4][:, 256 * (i % 4) : 256 * (i % 4 + 1)]

                # ---- Q^T packed: QT_sb[:, 256*j + 128*k] ----
                pq = ps_m.tile([128, 512], bf16, tag="m")
                for j in range(NMT):
                    for k in range(KH):
                        nc.tensor.transpose(
                            pq[:, 256 * j + 128 * k : 256 * j + 128 * (k + 1)],
                            Q_bf[k][:, 128 * j : 128 * (j + 1)],
                            identity_bf,
                        )
                QT_sb = sb3.tile([128, 512], bf16, tag="qt")
                nc.vector.tensor_copy(QT_sb, pq)

                st.update(
                    C_bf=C_bf, CAs=CAs, r_col=r_col, CtT=CtT, QT_sb=QT_sb
                )
                return st

            def backend(b, st):
                C_nat = st["C_nat"]; Q_nat = st["Q_nat"]; C_bf = st["C_bf"]
                CAs = st["CAs"]; r_col = st["r_col"]; CtT = st["CtT"]
                QT_sb = st["QT_sb"]

                # ---- Z = exp(S) in [l, m] + rowsums rho ----
                rho_col = sb.tile([128, NLT], f32, tag="rho")
                E_lm = []
                for i in range(NLT):
                    pz = ps_m.tile([128, LQ], f32, tag="pz")
                    for k in range(KH):
                        mm(
                            pz,
                            CAs(k, i),
                            Q_nat[k],
                            (k == 0),
                            (k == KH - 1),
                        )
                    e = sb.tile([128, LQ], bf16, tag=f"elm{i}")
                    nc.scalar.activation(
                        out=e,
                        in_=pz,
                        func=AF.Exp,
                        bias=r_col[:, i : i + 1],
                        accum_out=rho_col[:, i : i + 1],
                    )
                    E_lm.append(e)

                # ---- per-tile rho_inv + diag(rho_inv) matrices ----
                # D[i] = identity * rho_inv[:, i] lets the P^T "transpose"
                # run as a plain matmul E_lm[i]^T @ D[i], fusing the row-softmax
                # scaling into the transpose for free.
                rho_inv = sb.tile([128, NLT], f32, tag="rhoi")
                D = []
                for i in range(NLT):
                    nc.vector.reciprocal(
                        rho_inv[:, i : i + 1], rho_col[:, i : i + 1]
                    )
                    d = sb.tile([128, 128], bf16, tag=f"diag{i}")
                    nc.gpsimd.tensor_scalar_mul(
                        d, identity_bf, rho_inv[:, i : i + 1]
                    )
                    D.append(d)

                # ---- T~ = Z^T-contraction with Ct, + kappa via ones-mms ----
                kap_ps = ps_m.tile([128, NMT], f32, tag="m")
                T_psum = []
                for j in range(NMT):
                    pT = ps_m.tile([128, H], f32, tag="m")
                    for i in range(NLT):
                        mmb(
                            pT,
                            E_lm[i][:, 128 * j : 128 * (j + 1)],
                            CtT(i),
                            (i == 0),
                            (i == NLT - 1),
                        )
                        mmb(
                            kap_ps[:, j : j + 1],
                            E_lm[i][:, 128 * j : 128 * (j + 1)],
                            ones_col,
                            (i == 0),
                            (i == NLT - 1),
                        )
                    T_psum.append(pT)
                kap_inv = sb.tile([128, NMT], f32, tag="kapi")
                nc.vector.reciprocal(kap_inv, kap_ps)
                T_nat = []
                for j in range(NMT):
                    t = sb.tile([128, H], bf16, tag=f"tn{j}")
                    nc.vector.tensor_scalar_mul(t, T_psum[j], kap_inv[:, j : j + 1])
                    T_nat.append(t)

                # ---- P^T packed: P_ml[j][:, 128i+p] = Z[128i+p, 128j+m]*rho_inv
                #      via matmul E_lm[i]^T @ D[i] (scaled transpose) ----
                P_ml = []
                for j in range(NMT):
                    t = sb.tile([128, LC], bf16, tag=f"pml{j}")
                    for c in range(2):
                        pp = ps_g.tile([128, 512], f32, tag="g")
                        for q in range(4):
                            i = 4 * c + q
                            mmb(
                                pp[:, 128 * q : 128 * (q + 1)],
                                E_lm[i][:, 128 * j : 128 * (j + 1)],
                                D[i],
                                True,
                                True,
                            )
                        nc.vector.tensor_copy(
                            out=t[:, 512 * c : 512 * (c + 1)], in_=pp
                        )
                    P_ml.append(t)

                # ---- A^T and Bv^T  [h, l] ----
                A_T = []
                Bv_T = []
                for t_i in range(KH):
                    pA = ps_g.tile([128, 512], f32, tag="g")
                    pA2 = ps_g.tile([128, 512], f32, tag="g")
                    for c, pp in enumerate((pA, pA2)):
                        for j in range(NMT):
                            mmb(
                                pp,
                                QT_sb[:, 256 * j + 128 * t_i : 256 * j + 128 * (t_i + 1)],
                                P_ml[j][:, 512 * c : 512 * (c + 1)],
                                (j == 0),
                                (j == NMT - 1),
                            )
                    a = sb.tile([128, LC], bf16, tag=f"at{t_i}")
                    nc.vector.tensor_copy(out=a[:, 0:512], in_=pA)
                    nc.vector.tensor_copy(out=a[:, 512:1024], in_=pA2)
                    A_T.append(a)
                for t_i in range(KH):
                    pB = ps_g.tile([128, 512], f32, tag="g")
                    pB2 = ps_g.tile([128, 512], f32, tag="g")
                    for c, pp in enumerate((pB, pB2)):
                        for j in range(NMT):
                            mmb(
                                pp,
                                T_nat[j][:, 128 * t_i : 128 * (t_i + 1)],
                                P_ml[j][:, 512 * c : 512 * (c + 1)],
                                (j == 0),
                                (j == NMT - 1),
                            )
                    bv = sb.tile([128, LC], bf16, tag=f"bvt{t_i}")
                    nc.scalar.copy(out=bv[:, 0:512], in_=pB)
                    nc.scalar.copy(out=bv[:, 512:1024], in_=pB2)
                    Bv_T.append(bv)

                # ---- products (DVE bf16, per 512-chunk so final(c=0) starts early) ----
                CA1 = []
                CB1 = []
                for t_i in range(KH):
                    p1 = sb.tile([128, LC], bf16, tag=f"ca1{t_i}")
                    p2 = sb.tile([128, LC], bf16, tag=f"cb1{t_i}")
                    for c in range(2):
                        sl = slice(512 * c, 512 * (c + 1))
                        nc.vector.tensor_mul(p1[:, sl], C_bf[t_i][:, sl], A_T[t_i][:, sl])
                        nc.vector.tensor_mul(p2[:, sl], C_bf[t_i][:, sl], Bv_T[t_i][:, sl])
                    CA1.append(p1)
                    CB1.append(p2)

                # ---- final matmul + relu + store (bf16 out) ----
                blocks = [
                    C_bf[0],
                    C_bf[1],
                    A_T[0],
                    A_T[1],
                    CA1[0],
                    CA1[1],
                    CB1[0],
                    CB1[1],
                ]
                for t_i in range(KH):
                    for c in range(2):
                        po = ps_g.tile([128, 512], f32, tag="g")
                        for f in range(8):
                            mmb(
                                po,
                                WT(f)[:, 128 * t_i : 128 * (t_i + 1)],
                                blocks[f][:, 512 * c : 512 * (c + 1)],
                                (f == 0),
                                (f == 7),
                            )
                        o = sb.tile([128, 512], bf16, tag=f"osb{t_i}{c}")
                        nc.scalar.activation(
                            out=o,
                            in_=po,
                            func=AF.Relu,
                            bias=b_col[:, t_i : t_i + 1],
                        )
                        nc.sync.dma_start(
                            out=out.ap()[
                                b, 128 * t_i : 128 * (t_i + 1), 512 * c : 512 * (c + 1)
                            ],
                            in_=o,
                        )

            def body(iv=None):
                # All input DMAs issue up front (dedicated BB-deep pool) so no
                # load ever queues behind an output store on the DMA track.
                # W_res DMAs slot in right after batch 0's loads.
                st = [load(0)]
                if not _Wn:
                    load_W()
                for b in range(1, BB):
                    st.append(load(b))
                frontend(0, st[0])
                frontend(1, st[1])
                if not WT_sb:
                    build_WT()
                backend(0, st[0])
                frontend(2, st[2])
                backend(1, st[1])
                frontend(3, st[3])
                backend(2, st[2])
                backend(3, st[3])

            if reps == 1:
                body()
            else:
                # PE body is ~1.1k instructions (>4 IRAM blocks): hint the
                # back-edge so each iteration doesn't stall on an I$ refetch.
                with tc.For_i(
                    0, reps, 1, hint_engines=(mybir.EngineType.PE,)
                ) as iv:
                    body(iv)

    nc.compile()
    return nc


def _get_nc(reps: int = 1):
    key = ("nc", reps)
    if key not in _CACHE:
        _CACHE[key] = _build(reps)
    return _CACHE[key]


def kernel(C, Q, cmask, qmask, w, W_res, b_res, _reps: int = 1, _want_res: bool = False):
    from concourse.bass_utils import run_bass_kernel_spmd

    nc = _get_nc(_reps)

    C = np.ascontiguousarray(C, dtype=np.float32)
    Q = np.ascontiguousarray(Q, dtype=np.float32)
    w = np.ascontiguousarray(w, dtype=np.float32)
    W_res = np.ascontiguousarray(W_res, dtype=np.float32)
    b_res = np.ascontiguousarray(b_res, dtype=np.float32)

    in_maps = []
    for i in range(N_CORES):
        sl = slice(i * BB, (i + 1) * BB)
        in_maps.append(
            {"C": C[sl], "Q": Q[sl], "w": w, "W_res": W_res, "b_res": b_res}
        )

    res = run_bass_kernel_spmd(nc, in_maps, core_ids=list(range(N_CORES)))
    outs = [
        np.asarray(res.results[i]["out"]).astype(np.float32) for i in range(N_CORES)
    ]
    out = np.concatenate(outs, axis=0)
    if _want_res:
        return out, res
    return out


# revision 6
# speedup vs baseline: 3.3513x; 1.3197x over previous
"""CQAttention Trainium2 kernel — data-parallel over batch across 8 NeuronCores.

Problem shapes (hardcoded): B=32, H=256, Lc=1024, Lq=256.
Each core processes B/8 = 4 batches.

Math (per batch, all-ones masks guaranteed by the problem spec):
  Ct = C^T [Lc,H], Qt = Q^T [Lq,H]
  S[l,m] = Ct[l]@w1 + Qt[m]@w2 + (Ct[l]*w3)@Qt[m]
  Z = exp(S) serves BOTH softmaxes:
    S_row = Z / rowsum(Z),  S_col = Z / colsum(Z)
  A  = S_row @ Qt
  Bv = S_row @ (S_col^T @ Ct)      (factored: avoids the Lc x Lc product)
  out = relu([Ct, A, Ct*A, Ct*Bv] @ W_res^T + b_res)^T  -> [H, Lc]

Key structure (v2 — engine-load-balanced single-Z design):
  - Z is computed ONCE in [l,m] layout: pz = CA^T @ Q (+ q folded into
    CA = C*w3 + w2 along the free dim) + r via a K=1 matmul (lhsT = r_row
    slice, rhs = ones row) accumulated into the same PSUM — no DRAM
    bounce, no exp bias needed.
  - exp(pz) -> E_lm bf16 with accum_out giving rowsums rho for free.
  - Row-softmax weights P = E_lm * rho_inv (per-partition scalar on
    GpSimd), then PE-transposed into P_ml [m,l] (replaces the second
    Z^T matmul + exp of the baseline).
  - kappa (colsums of Z) via N=1 ones-matmuls sharing the E_lm lhsT of
    the T matmuls -> lands directly in [m-partition] column layout.
  - All PE transposes pack into wide PSUM tiles -> one DVE copy per
    1024 columns instead of per-128 copies.
  - 1-input elementwise (casts, CA, P scaling) runs on the otherwise
    idle GpSimd engine; DVE keeps PSUM reads and 2-input muls; ACT does
    only exp and the final bias+relu.
  - Output is stored bf16 (halves store traffic); converted to f32 on host.
"""

import numpy as np

_CACHE = {}

B_FULL = 32
N_CORES = 8
BB = B_FULL // N_CORES  # batches per core = 4
H = 256
LC = 1024
LQ = 256


def _build(reps: int = 1):
    from contextlib import ExitStack

    import concourse.bass as bass
    import concourse.tile as tile
    from concourse import bacc, mybir
    from concourse.masks import make_identity

    f32 = mybir.dt.float32
    f32r = mybir.dt.float32r
    bf16 = mybir.dt.bfloat16
    AF = mybir.ActivationFunctionType
    OP = mybir.AluOpType

    nc = bacc.Bacc("TRN2", target_bir_lowering=False, debug=False)

    def mm(out, lhsT, rhs, start, stop):
        # fp32r runs the PE at full rate (1 cycle/row for N>=256)
        nc.tensor.matmul(
            out,
            lhsT=lhsT.bitcast(f32r),
            rhs=rhs.bitcast(f32r),
            start=start,
            stop=stop,
        )

    def mmb(out, lhsT, rhs, start, stop):
        nc.tensor.matmul(out, lhsT=lhsT, rhs=rhs, start=start, stop=stop)

    C = nc.dram_tensor("C", [BB, H, LC], f32, kind="ExternalInput")
    Q = nc.dram_tensor("Q", [BB, H, LQ], f32, kind="ExternalInput")
    w = nc.dram_tensor("w", [3 * H], f32, kind="ExternalInput")
    W_res = nc.dram_tensor("W_res", [H, 4 * H], f32, kind="ExternalInput")
    b_res = nc.dram_tensor("b_res", [H], f32, kind="ExternalInput")
    out = nc.dram_tensor("out", [BB, H, LC], bf16, kind="ExternalOutput")

    KH = H // 128  # 2 h-chunks
    NLT = LC // 128  # 8 l-tiles
    NMT = LQ // 128  # 2 m-tiles

    with tile.TileContext(nc) as tc:
        with ExitStack() as ctx:
            singles = ctx.enter_context(tc.tile_pool(name="singles", bufs=1))
            sb = ctx.enter_context(tc.tile_pool(name="sb", bufs=2))
            sb3 = ctx.enter_context(tc.tile_pool(name="sb3", bufs=3))
            sbL = ctx.enter_context(tc.tile_pool(name="sbL", bufs=BB))
            ps_m = ctx.enter_context(
                tc.tile_pool(name="ps_m", bufs=2, space="PSUM")
            )
            ps_g = ctx.enter_context(
                tc.tile_pool(name="ps_g", bufs=4, space="PSUM")
            )

            # ---- one-time constants ----
            identity_bf = singles.tile([128, 128], bf16)
            make_identity(nc, identity_bf)

            # w1 feeds only the tiny N=1 r-matmuls, which run in plain fp32
            # (fp32r has ISA restrictions at small free dims)
            w1_col = singles.tile([128, KH], f32)
            nc.sync.dma_start(
                out=w1_col,
                in_=w.ap()[0:H].rearrange("(i p) -> p i", i=KH, p=128),
            )
            w23 = singles.tile([128, 2 * KH], f32)
            nc.sync.dma_start(
                out=w23,
                in_=w.ap()[H : 3 * H].rearrange("(i p) -> p i", i=2 * KH, p=128),
            )
            w2_col = w23[:, 0:KH]
            w3_col = w23[:, KH : 2 * KH]
            b_col = singles.tile([128, KH], f32)
            nc.sync.dma_start(
                out=b_col, in_=b_res.ap().rearrange("(i p) -> p i", i=KH, p=128)
            )

            # persistent C^T slots (2 per batch, 3-deep rotation); the ones
            # columns at 257-stride are written once and never overwritten.
            ctt_slots = []
            for s in range(6):
                t_ct = singles.tile([128, 4 * 257], bf16, tag=f"ctts{s}")
                tv = t_ct.rearrange("p (q c) -> p q c", q=4, c=257)
                nc.vector.memset(tv[:, :, 256:257], 1.0)
                ctt_slots.append(t_ct)

            WT_sb = []
            _Wn = []

            def load_W():
                # W_res loads + bf16 casts (issued early; transposes come later)
                for j in range(KH):
                    t = singles.tile([128, 4 * H], f32, tag=f"wn{j}")
                    nc.sync.dma_start(
                        out=t, in_=W_res.ap()[128 * j : 128 * (j + 1), :]
                    )
                    tb = singles.tile([128, 4 * H], bf16, tag=f"wnb{j}")
                    nc.gpsimd.tensor_copy(tb, t)
                    _Wn.append(tb)

            def build_WT():
                # W_res^T (bf16), packed: WT_sb[g][:, 256*fl + 128*j] holds
                # W_res[128j:128j+128, 128*(4g+fl):...]^T — own PSUM tag so the
                # one-time build never blocks the per-batch "m" slot rotation.
                for g in range(2):
                    # [128,1024] bf16 = same bytes as the [128,512] f32 "g"
                    # slots, so this borrows the ps_g pool without growing it.
                    pw = ps_g.tile([128, 1024], bf16, tag="g")
                    for fl in range(4):
                        f = 4 * g + fl
                        for j in range(KH):
                            nc.tensor.transpose(
                                pw[:, 256 * fl + 128 * j : 256 * fl + 128 * (j + 1)],
                                _Wn[j][:, 128 * f : 128 * (f + 1)],
                                identity_bf,
                            )
                    t_wt = singles.tile([128, 1024], bf16, tag=f"wt{g}")
                    nc.vector.tensor_copy(t_wt, pw)
                    WT_sb.append(t_wt)

            def WT(f):  # [128 (f-part), H]
                return WT_sb[f // 4][:, 256 * (f % 4) : 256 * (f % 4 + 1)]

            NCH = 1  # C column chunks (finer DMA granularity for the ramp)
            CW = LC // NCH
            TPC = NLT // NCH  # l-tiles per chunk

            def load(b):
                # C loads in NCH column chunks per k so downstream per-chunk
                # casts/CA/E-matmuls start as soon as the first chunk lands.
                C_nat = []
                Q_nat = []
                for k in range(KH):
                    chunks = []
                    for h in range(NCH):
                        t = sbL.tile([128, CW], f32r, tag=f"cnat{k}_{h}")
                        nc.sync.dma_start(
                            out=t,
                            in_=C.ap()[
                                b, 128 * k : 128 * (k + 1), CW * h : CW * (h + 1)
                            ].bitcast(f32r),
                        )
                        chunks.append(t)
                    C_nat.append(chunks)
                    tq = sbL.tile([128, LQ], f32r, tag=f"qnat{k}")
                    nc.sync.dma_start(
                        out=tq,
                        in_=Q.ap()[b, 128 * k : 128 * (k + 1), :].bitcast(f32r),
                    )
                    Q_nat.append(tq)
                return {"C_nat": C_nat, "Q_nat": Q_nat}

            def frontend(b, st):
                C_nat = st["C_nat"]
                Q_nat = st["Q_nat"]
                # ---- bf16 copies + CA = C*w3 + w2 (GpSimd, per chunk) ----
                C_bf = []
                Q_bf = []
                CA = []
                for k in range(KH):
                    cb = sb.tile([128, LC], bf16, tag=f"cbf{k}")
                    ca_ch = []
                    for h in range(NCH):
                        t = sb.tile([128, CW], f32r, tag=f"ca{k}_{h}")
                        nc.gpsimd.tensor_scalar(
                            out=t,
                            in0=C_nat[k][h].bitcast(f32),
                            scalar1=w3_col[:, k : k + 1],
                            scalar2=w2_col[:, k : k + 1],
                            op0=OP.mult,
                            op1=OP.add,
                        )
                        ca_ch.append(t)
                        nc.gpsimd.tensor_copy(
                            cb[:, CW * h : CW * (h + 1)], C_nat[k][h].bitcast(f32)
                        )
                    CA.append(ca_ch)
                    C_bf.append(cb)
                    qb = sb.tile([128, LQ], bf16, tag=f"qbf{k}")
                    nc.gpsimd.tensor_copy(qb, Q_nat[k].bitcast(f32))
                    Q_bf.append(qb)

                def CAs(k, i):  # [128, 128] slice of CA for l-tile i
                    return CA[k][i // TPC][:, 128 * (i % TPC) : 128 * (i % TPC + 1)]

                # ---- r_col[p, i] = sum_h C[h, 128i+p] w1[h] (N=1 matmuls):
                #      lands directly in the per-partition layout exp's bias wants.
                r_ps = ps_m.tile([128, NLT], f32, tag="m")
                for i in range(NLT):
                    for k in range(KH):
                        nc.tensor.matmul(
                            r_ps[:, i : i + 1],
                            lhsT=C_nat[k][i // TPC][
                                :, 128 * (i % TPC) : 128 * (i % TPC + 1)
                            ].bitcast(f32),
                            rhs=w1_col[:, k : k + 1],
                            start=(k == 0),
                            stop=(k == KH - 1),
                        )
                r_col = sb.tile([128, NLT], f32, tag="rcol")
                nc.vector.tensor_copy(r_col, r_ps)

                # ---- C^T packed with a ones column per l-tile (257-stride):
                # CtT_ext(i) = [Ct rows | 1] so the T matmuls emit kappa
                # (colsums of Z) as a free 257th output column.
                CtT_sb = []
                for p in range(2):
                    pt = ps_m.tile([128, 1024], bf16, tag="m")
                    for q in range(4):
                        i = 4 * p + q
                        for k in range(KH):
                            nc.tensor.transpose(
                                pt[:, 256 * q + 128 * k : 256 * q + 128 * (k + 1)],
                                C_bf[k][:, 128 * i : 128 * (i + 1)],
                                identity_bf,
                            )
                    t_ct = ctt_slots[(2 * b + p) % 6]
                    # strided copy: psum [128, 4, 256] -> sbuf stride-257 blocks
                    tv = t_ct.rearrange("p (q c) -> p q c", q=4, c=257)
                    nc.vector.tensor_copy(
                        out=tv[:, :, 0:256],
                        in_=pt.rearrange("p (q c) -> p q c", q=4, c=256),
                    )
                    CtT_sb.append(t_ct)

                def CtT(i):
                    return CtT_sb[i // 4][:, 257 * (i % 4) : 257 * (i % 4) + 257]

                # ---- Q^T packed: QT_sb[:, 256*j + 128*k] ----
                pq = ps_m.tile([128, 512], bf16, tag="m")
                for j in range(NMT):
                    for k in range(KH):
                        nc.tensor.transpose(
                            pq[:, 256 * j + 128 * k : 256 * j + 128 * (k + 1)],
                            Q_bf[k][:, 128 * j : 128 * (j + 1)],
                            identity_bf,
                        )
                QT_sb = sb3.tile([128, 512], bf16, tag="qt")
                nc.vector.tensor_copy(QT_sb, pq)

                st.update(
                    C_bf=C_bf, CAs=CAs, r_col=r_col, CtT=CtT, QT_sb=QT_sb
                )
                return st

            def backend(b, st):
                C_nat = st["C_nat"]; Q_nat = st["Q_nat"]; C_bf = st["C_bf"]
                CAs = st["CAs"]; r_col = st["r_col"]; CtT = st["CtT"]
                QT_sb = st["QT_sb"]

                # ---- Z = exp(S) in [l, m] + rowsums rho ----
                rho_col = sb.tile([128, NLT], f32, tag="rho")
                E_lm = []
                for i in range(NLT):
                    pz = ps_m.tile([128, LQ], f32, tag="pz")
                    for k in range(KH):
                        mm(
                            pz,
                            CAs(k, i),
                            Q_nat[k],
                            (k == 0),
                            (k == KH - 1),
                        )
                    e = sb.tile([128, LQ], bf16, tag=f"elm{i}")
                    nc.scalar.activation(
                        out=e,
                        in_=pz,
                        func=AF.Exp,
                        bias=r_col[:, i : i + 1],
                        accum_out=rho_col[:, i : i + 1],
                    )
                    E_lm.append(e)

                # ---- rho_inv + all 8 diag(rho_inv) blocks in one Pool op:
                # D_all[p, 128i+c] = identity[p,c] * rho_inv[p,i]. The P^T
                # "transpose" then runs as a plain matmul E_lm[i]^T @ D(i),
                # fusing the row-softmax scaling into the transpose for free.
                rho_inv = sb.tile([128, NLT], f32, tag="rhoi")
                nc.vector.reciprocal(rho_inv, rho_col)
                D_all = sb.tile([128, NLT * 128], bf16, tag="diag")
                ident_b = bass.AP(
                    tensor=identity_bf.tensor,
                    offset=identity_bf.offset,
                    ap=[list(identity_bf.ap[0]), [0, NLT], [1, 128]],
                )
                rho_b = bass.AP(
                    tensor=rho_inv.tensor,
                    offset=rho_inv.offset,
                    ap=[list(rho_inv.ap[0]), [1, NLT], [0, 128]],
                )
                nc.gpsimd.tensor_tensor(
                    out=D_all, in0=ident_b, in1=rho_b, op=OP.mult
                )

                def D(i):
                    return D_all[:, 128 * i : 128 * (i + 1)]

                # ---- T~ = Z^T-contraction with Ct_ext; col 256 = kappa free ----
                T_psum = []
                for j in range(NMT):
                    pT = ps_m.tile([128, H + 1], f32, tag="m")
                    for i in range(NLT):
                        mmb(
                            pT,
                            E_lm[i][:, 128 * j : 128 * (j + 1)],
                            CtT(i),
                            (i == 0),
                            (i == NLT - 1),
                        )
                    T_psum.append(pT)
                kap_inv = sb.tile([128, NMT], f32, tag="kapi")
                for j in range(NMT):
                    nc.vector.reciprocal(
                        kap_inv[:, j : j + 1], T_psum[j][:, H : H + 1]
                    )
                T_nat = []
                for j in range(NMT):
                    t = sb.tile([128, H], bf16, tag=f"tn{j}")
                    nc.vector.tensor_scalar_mul(
                        t, T_psum[j][:, 0:H], kap_inv[:, j : j + 1]
                    )
                    T_nat.append(t)

                # ---- P^T packed: P_ml[j][:, 128i+p] = Z[128i+p, 128j+m]*rho_inv
                #      via matmul E_lm[i]^T @ D[i] (scaled transpose) ----
                P_ml = []
                for j in range(NMT):
                    t = sb.tile([128, LC], bf16, tag=f"pml{j}")
                    for c in range(2):
                        pp = ps_g.tile([128, 512], f32, tag="g")
                        for q in range(4):
                            i = 4 * c + q
                            mmb(
                                pp[:, 128 * q : 128 * (q + 1)],
                                E_lm[i][:, 128 * j : 128 * (j + 1)],
                                D(i),
                                True,
                                True,
                            )
                        nc.vector.tensor_copy(
                            out=t[:, 512 * c : 512 * (c + 1)], in_=pp
                        )
                    P_ml.append(t)

                # ---- A^T and Bv^T  [h, l] ----
                A_T = []
                Bv_T = []
                for t_i in range(KH):
                    pA = ps_g.tile([128, 512], f32, tag="g")
                    pA2 = ps_g.tile([128, 512], f32, tag="g")
                    for c, pp in enumerate((pA, pA2)):
                        for j in range(NMT):
                            mmb(
                                pp,
                                QT_sb[:, 256 * j + 128 * t_i : 256 * j + 128 * (t_i + 1)],
                                P_ml[j][:, 512 * c : 512 * (c + 1)],
                                (j == 0),
                                (j == NMT - 1),
                            )
                    a = sb.tile([128, LC], bf16, tag=f"at{t_i}")
                    nc.vector.tensor_copy(out=a[:, 0:512], in_=pA)
                    nc.vector.tensor_copy(out=a[:, 512:1024], in_=pA2)
                    A_T.append(a)
                for t_i in range(KH):
                    pB = ps_g.tile([128, 512], f32, tag="g")
                    pB2 = ps_g.tile([128, 512], f32, tag="g")
                    for c, pp in enumerate((pB, pB2)):
                        for j in range(NMT):
                            mmb(
                                pp,
                                T_nat[j][:, 128 * t_i : 128 * (t_i + 1)],
                                P_ml[j][:, 512 * c : 512 * (c + 1)],
                                (j == 0),
                                (j == NMT - 1),
                            )
                    bv = sb.tile([128, LC], bf16, tag=f"bvt{t_i}")
                    nc.scalar.copy(out=bv[:, 0:512], in_=pB)
                    nc.scalar.copy(out=bv[:, 512:1024], in_=pB2)
                    Bv_T.append(bv)

                # ---- products (DVE bf16, per 512-chunk so final(c=0) starts early) ----
                CA1 = []
                CB1 = []
                for t_i in range(KH):
                    p1 = sb.tile([128, LC], bf16, tag=f"ca1{t_i}")
                    p2 = sb.tile([128, LC], bf16, tag=f"cb1{t_i}")
                    for c in range(2):
                        sl = slice(512 * c, 512 * (c + 1))
                        nc.vector.tensor_mul(p1[:, sl], C_bf[t_i][:, sl], A_T[t_i][:, sl])
                        nc.vector.tensor_mul(p2[:, sl], C_bf[t_i][:, sl], Bv_T[t_i][:, sl])
                    CA1.append(p1)
                    CB1.append(p2)

                # ---- final matmul + relu + store (bf16 out) ----
                blocks = [
                    C_bf[0],
                    C_bf[1],
                    A_T[0],
                    A_T[1],
                    CA1[0],
                    CA1[1],
                    CB1[0],
                    CB1[1],
                ]
                for t_i in range(KH):
                    for c in range(2):
                        po = ps_g.tile([128, 512], f32, tag="g")
                        for f in range(8):
                            mmb(
                                po,
                                WT(f)[:, 128 * t_i : 128 * (t_i + 1)],
                                blocks[f][:, 512 * c : 512 * (c + 1)],
                                (f == 0),
                                (f == 7),
                            )
                        o = sb.tile([128, 512], bf16, tag=f"osb{t_i}{c}")
                        nc.scalar.activation(
                            out=o,
                            in_=po,
                            func=AF.Relu,
                            bias=b_col[:, t_i : t_i + 1],
                        )
                        nc.sync.dma_start(
                            out=out.ap()[
                                b, 128 * t_i : 128 * (t_i + 1), 512 * c : 512 * (c + 1)
                            ],
                            in_=o,
                        )

            def body(iv=None):
                # All input DMAs issue up front (dedicated BB-deep pool) so no
                # load ever queues behind an output store on the DMA track.
                # W_res DMAs slot in right after batch 0's loads.
                st = [load(0)]
                if not _Wn:
                    load_W()
                for b in range(1, BB):
                    st.append(load(b))
                frontend(0, st[0])
                frontend(1, st[1])
                if not WT_sb:
                    build_WT()
                backend(0, st[0])
                frontend(2, st[2])
                backend(1, st[1])
                frontend(3, st[3])
                backend(2, st[2])
                backend(3, st[3])

            if reps == 1:
                body()
            else:
                # PE body is ~1.1k instructions (>4 IRAM blocks): hint the
                # back-edge so each iteration doesn't stall on an I$ refetch.
                with tc.For_i(
                    0, reps, 1, hint_engines=(mybir.EngineType.PE,)
                ) as iv:
                    body(iv)

    nc.compile()
    return nc


def _get_nc(reps: int = 1):
    key = ("nc", reps)
    if key not in _CACHE:
        _CACHE[key] = _build(reps)
    return _CACHE[key]


def kernel(C, Q, cmask, qmask, w, W_res, b_res, _reps: int = 1, _want_res: bool = False):
    from concourse.bass_utils import run_bass_kernel_spmd

    nc = _get_nc(_reps)

    C = np.ascontiguousarray(C, dtype=np.float32)
    Q = np.ascontiguousarray(Q, dtype=np.float32)
    w = np.ascontiguousarray(w, dtype=np.float32)
    W_res = np.ascontiguousarray(W_res, dtype=np.float32)
    b_res = np.ascontiguousarray(b_res, dtype=np.float32)

    in_maps = []
    for i in range(N_CORES):
        sl = slice(i * BB, (i + 1) * BB)
        in_maps.append(
            {"C": C[sl], "Q": Q[sl], "w": w, "W_res": W_res, "b_res": b_res}
        )

    res = run_bass_kernel_spmd(nc, in_maps, core_ids=list(range(N_CORES)))
    outs = [
        np.asarray(res.results[i]["out"]).astype(np.float32) for i in range(N_CORES)
    ]
    out = np.concatenate(outs, axis=0)
    if _want_res:
        return out, res
    return out


# revision 7
# speedup vs baseline: 3.6006x; 1.0744x over previous
"""CQAttention Trainium2 kernel — data-parallel over batch across 8 NeuronCores.

Problem shapes (hardcoded): B=32, H=256, Lc=1024, Lq=256.
Each core processes B/8 = 4 batches.

Math (per batch, all-ones masks guaranteed by the problem spec):
  Ct = C^T [Lc,H], Qt = Q^T [Lq,H]
  S[l,m] = Ct[l]@w1 + Qt[m]@w2 + (Ct[l]*w3)@Qt[m]
  Z = exp(S) serves BOTH softmaxes:
    S_row = Z / rowsum(Z),  S_col = Z / colsum(Z)
  A  = S_row @ Qt
  Bv = S_row @ (S_col^T @ Ct)      (factored: avoids the Lc x Lc product)
  out = relu([Ct, A, Ct*A, Ct*Bv] @ W_res^T + b_res)^T  -> [H, Lc]

Key structure (v2 — engine-load-balanced single-Z design):
  - Z is computed ONCE in [l,m] layout: pz = CA^T @ Q (+ q folded into
    CA = C*w3 + w2 along the free dim) + r via a K=1 matmul (lhsT = r_row
    slice, rhs = ones row) accumulated into the same PSUM — no DRAM
    bounce, no exp bias needed.
  - exp(pz) -> E_lm bf16 with accum_out giving rowsums rho for free.
  - Row-softmax weights P = E_lm * rho_inv (per-partition scalar on
    GpSimd), then PE-transposed into P_ml [m,l] (replaces the second
    Z^T matmul + exp of the baseline).
  - kappa (colsums of Z) via N=1 ones-matmuls sharing the E_lm lhsT of
    the T matmuls -> lands directly in [m-partition] column layout.
  - All PE transposes pack into wide PSUM tiles -> one DVE copy per
    1024 columns instead of per-128 copies.
  - 1-input elementwise (casts, CA, P scaling) runs on the otherwise
    idle GpSimd engine; DVE keeps PSUM reads and 2-input muls; ACT does
    only exp and the final bias+relu.
  - Output is stored bf16 (halves store traffic); converted to f32 on host.
"""

import numpy as np

_CACHE = {}

B_FULL = 32
N_CORES = 8
BB = B_FULL // N_CORES  # batches per core = 4
H = 256
LC = 1024
LQ = 256


def _build(reps: int = 1):
    from contextlib import ExitStack

    import concourse.bass as bass
    import concourse.tile as tile
    from concourse import bacc, mybir
    from concourse.masks import make_identity

    f32 = mybir.dt.float32
    f32r = mybir.dt.float32r
    bf16 = mybir.dt.bfloat16
    AF = mybir.ActivationFunctionType
    OP = mybir.AluOpType

    nc = bacc.Bacc("TRN2", target_bir_lowering=False, debug=False)

    def mm(out, lhsT, rhs, start, stop):
        # fp32r runs the PE at full rate (1 cycle/row for N>=256)
        nc.tensor.matmul(
            out,
            lhsT=lhsT.bitcast(f32r),
            rhs=rhs.bitcast(f32r),
            start=start,
            stop=stop,
        )

    def mmb(out, lhsT, rhs, start, stop):
        nc.tensor.matmul(out, lhsT=lhsT, rhs=rhs, start=start, stop=stop)

    C = nc.dram_tensor("C", [BB, H, LC], f32, kind="ExternalInput")
    Q = nc.dram_tensor("Q", [BB, H, LQ], f32, kind="ExternalInput")
    w = nc.dram_tensor("w", [3 * H], f32, kind="ExternalInput")
    W_res = nc.dram_tensor("W_res", [H, 4 * H], f32, kind="ExternalInput")
    b_res = nc.dram_tensor("b_res", [H], f32, kind="ExternalInput")
    out = nc.dram_tensor("out", [BB, H, LC], bf16, kind="ExternalOutput")

    KH = H // 128  # 2 h-chunks
    NLT = LC // 128  # 8 l-tiles
    NMT = LQ // 128  # 2 m-tiles

    with tile.TileContext(nc) as tc:
        with ExitStack() as ctx:
            singles = ctx.enter_context(tc.tile_pool(name="singles", bufs=1))
            sb = ctx.enter_context(tc.tile_pool(name="sb", bufs=2))
            sb3 = ctx.enter_context(tc.tile_pool(name="sb3", bufs=3))
            sbL = ctx.enter_context(tc.tile_pool(name="sbL", bufs=BB))
            ps_m = ctx.enter_context(
                tc.tile_pool(name="ps_m", bufs=2, space="PSUM")
            )
            ps_g = ctx.enter_context(
                tc.tile_pool(name="ps_g", bufs=4, space="PSUM")
            )

            # ---- one-time constants ----
            identity_bf = singles.tile([128, 128], bf16)
            make_identity(nc, identity_bf)

            # w1 feeds only the tiny N=1 r-matmuls, which run in plain fp32
            # (fp32r has ISA restrictions at small free dims)
            w1_col = singles.tile([128, KH], f32)
            nc.sync.dma_start(
                out=w1_col,
                in_=w.ap()[0:H].rearrange("(i p) -> p i", i=KH, p=128),
            )
            w23 = singles.tile([128, 2 * KH], f32)
            nc.sync.dma_start(
                out=w23,
                in_=w.ap()[H : 3 * H].rearrange("(i p) -> p i", i=2 * KH, p=128),
            )
            w2_col = w23[:, 0:KH]
            w3_col = w23[:, KH : 2 * KH]
            b_col = singles.tile([128, KH], f32)
            nc.sync.dma_start(
                out=b_col, in_=b_res.ap().rearrange("(i p) -> p i", i=KH, p=128)
            )

            # persistent C^T slots (2 per batch, 3-deep rotation); the ones
            # columns at 257-stride are written once and never overwritten.
            ctt_slots = []
            for s in range(6):
                t_ct = singles.tile([128, 4 * 257], bf16, tag=f"ctts{s}")
                tv = t_ct.rearrange("p (q c) -> p q c", q=4, c=257)
                nc.vector.memset(tv[:, :, 256:257], 1.0)
                ctt_slots.append(t_ct)

            WT_sb = []
            _Wn = []

            def load_W():
                # W_res loads + bf16 casts (issued early; transposes come later)
                for j in range(KH):
                    t = singles.tile([128, 4 * H], f32, tag=f"wn{j}")
                    nc.sync.dma_start(
                        out=t, in_=W_res.ap()[128 * j : 128 * (j + 1), :]
                    )
                    tb = singles.tile([128, 4 * H], bf16, tag=f"wnb{j}")
                    nc.gpsimd.tensor_copy(tb, t)
                    _Wn.append(tb)

            def build_WT():
                # W_res^T (bf16), packed: WT_sb[g][:, 256*fl + 128*j] holds
                # W_res[128j:128j+128, 128*(4g+fl):...]^T — own PSUM tag so the
                # one-time build never blocks the per-batch "m" slot rotation.
                for g in range(2):
                    # [128,1024] bf16 = same bytes as the [128,512] f32 "g"
                    # slots, so this borrows the ps_g pool without growing it.
                    pw = ps_g.tile([128, 1024], bf16, tag="g")
                    for fl in range(4):
                        f = 4 * g + fl
                        for j in range(KH):
                            nc.tensor.transpose(
                                pw[:, 256 * fl + 128 * j : 256 * fl + 128 * (j + 1)],
                                _Wn[j][:, 128 * f : 128 * (f + 1)],
                                identity_bf,
                            )
                    t_wt = singles.tile([128, 1024], bf16, tag=f"wt{g}")
                    nc.vector.tensor_copy(t_wt, pw)
                    WT_sb.append(t_wt)

            def WT(f):  # [128 (f-part), H]
                return WT_sb[f // 4][:, 256 * (f % 4) : 256 * (f % 4 + 1)]

            NCH = 1  # C column chunks (finer DMA granularity for the ramp)
            CW = LC // NCH
            TPC = NLT // NCH  # l-tiles per chunk

            def load(b):
                # C loads in NCH column chunks per k so downstream per-chunk
                # casts/CA/E-matmuls start as soon as the first chunk lands.
                C_nat = []
                Q_nat = []
                for k in range(KH):
                    chunks = []
                    for h in range(NCH):
                        t = sbL.tile([128, CW], f32r, tag=f"cnat{k}_{h}")
                        nc.sync.dma_start(
                            out=t,
                            in_=C.ap()[
                                b, 128 * k : 128 * (k + 1), CW * h : CW * (h + 1)
                            ].bitcast(f32r),
                        )
                        chunks.append(t)
                    C_nat.append(chunks)
                    tq = sbL.tile([128, LQ], f32r, tag=f"qnat{k}")
                    nc.sync.dma_start(
                        out=tq,
                        in_=Q.ap()[b, 128 * k : 128 * (k + 1), :].bitcast(f32r),
                    )
                    Q_nat.append(tq)
                return {"C_nat": C_nat, "Q_nat": Q_nat}

            def frontend(b, st):
                C_nat = st["C_nat"]
                Q_nat = st["Q_nat"]
                # ---- bf16 copies + CA = C*w3 + w2 (GpSimd, per chunk) ----
                C_bf = []
                Q_bf = []
                CA = []
                for k in range(KH):
                    cb = sb.tile([128, LC], bf16, tag=f"cbf{k}")
                    ca_ch = []
                    for h in range(NCH):
                        t = sb.tile([128, CW], f32r, tag=f"ca{k}_{h}")
                        nc.gpsimd.tensor_scalar(
                            out=t,
                            in0=C_nat[k][h].bitcast(f32),
                            scalar1=w3_col[:, k : k + 1],
                            scalar2=w2_col[:, k : k + 1],
                            op0=OP.mult,
                            op1=OP.add,
                        )
                        ca_ch.append(t)
                        nc.gpsimd.tensor_copy(
                            cb[:, CW * h : CW * (h + 1)], C_nat[k][h].bitcast(f32)
                        )
                    CA.append(ca_ch)
                    C_bf.append(cb)
                    qb = sb.tile([128, LQ], bf16, tag=f"qbf{k}")
                    nc.gpsimd.tensor_copy(qb, Q_nat[k].bitcast(f32))
                    Q_bf.append(qb)

                def CAs(k, i):  # [128, 128] slice of CA for l-tile i
                    return CA[k][i // TPC][:, 128 * (i % TPC) : 128 * (i % TPC + 1)]

                # ---- r_col[p, i] = sum_h C[h, 128i+p] w1[h] (N=1 matmuls):
                #      lands directly in the per-partition layout exp's bias wants.
                r_ps = ps_m.tile([128, NLT], f32, tag="m")
                for i in range(NLT):
                    for k in range(KH):
                        nc.tensor.matmul(
                            r_ps[:, i : i + 1],
                            lhsT=C_nat[k][i // TPC][
                                :, 128 * (i % TPC) : 128 * (i % TPC + 1)
                            ].bitcast(f32),
                            rhs=w1_col[:, k : k + 1],
                            start=(k == 0),
                            stop=(k == KH - 1),
                        )
                r_col = sb.tile([128, NLT], f32, tag="rcol")
                nc.vector.tensor_copy(r_col, r_ps)

                # ---- C^T packed with a ones column per l-tile (257-stride):
                # CtT_ext(i) = [Ct rows | 1] so the T matmuls emit kappa
                # (colsums of Z) as a free 257th output column.
                CtT_sb = []
                for p in range(2):
                    pt = ps_m.tile([128, 1024], bf16, tag="m")
                    for q in range(4):
                        i = 4 * p + q
                        for k in range(KH):
                            nc.tensor.transpose(
                                pt[:, 256 * q + 128 * k : 256 * q + 128 * (k + 1)],
                                C_bf[k][:, 128 * i : 128 * (i + 1)],
                                identity_bf,
                            )
                    t_ct = ctt_slots[(2 * b + p) % 6]
                    # strided copy: psum [128, 4, 256] -> sbuf stride-257 blocks
                    tv = t_ct.rearrange("p (q c) -> p q c", q=4, c=257)
                    nc.vector.tensor_copy(
                        out=tv[:, :, 0:256],
                        in_=pt.rearrange("p (q c) -> p q c", q=4, c=256),
                    )
                    CtT_sb.append(t_ct)

                def CtT(i):
                    return CtT_sb[i // 4][:, 257 * (i % 4) : 257 * (i % 4) + 257]

                # ---- Q^T packed: QT_sb[:, 256*j + 128*k] ----
                pq = ps_m.tile([128, 512], bf16, tag="m")
                for j in range(NMT):
                    for k in range(KH):
                        nc.tensor.transpose(
                            pq[:, 256 * j + 128 * k : 256 * j + 128 * (k + 1)],
                            Q_bf[k][:, 128 * j : 128 * (j + 1)],
                            identity_bf,
                        )
                QT_sb = sb3.tile([128, 512], bf16, tag="qt")
                nc.vector.tensor_copy(QT_sb, pq)

                st.update(
                    C_bf=C_bf, CAs=CAs, r_col=r_col, CtT=CtT, QT_sb=QT_sb
                )
                return st

            def backend(b, st):
                C_nat = st["C_nat"]; Q_nat = st["Q_nat"]; C_bf = st["C_bf"]
                CAs = st["CAs"]; r_col = st["r_col"]; CtT = st["CtT"]
                QT_sb = st["QT_sb"]

                # ---- Z = exp(S) in [l, m] + rowsums rho ----
                rho_col = sb.tile([128, NLT], f32, tag="rho")
                E_lm = []
                for i in range(NLT):
                    pz = ps_m.tile([128, LQ], f32, tag="pz")
                    for k in range(KH):
                        mm(
                            pz,
                            CAs(k, i),
                            Q_nat[k],
                            (k == 0),
                            (k == KH - 1),
                        )
                    e = sb.tile([128, LQ], bf16, tag=f"elm{i}")
                    nc.scalar.activation(
                        out=e,
                        in_=pz,
                        func=AF.Exp,
                        bias=r_col[:, i : i + 1],
                        accum_out=rho_col[:, i : i + 1],
                    )
                    E_lm.append(e)

                # ---- rho_inv + all 8 diag(rho_inv) blocks in one Pool op:
                # D_all[p, 128i+c] = identity[p,c] * rho_inv[p,i]. The P^T
                # "transpose" then runs as a plain matmul E_lm[i]^T @ D(i),
                # fusing the row-softmax scaling into the transpose for free.
                rho_inv = sb.tile([128, NLT], f32, tag="rhoi")
                nc.vector.reciprocal(rho_inv, rho_col)
                D_all = sb.tile([128, NLT * 128], bf16, tag="diag")
                ident_b = bass.AP(
                    tensor=identity_bf.tensor,
                    offset=identity_bf.offset,
                    ap=[list(identity_bf.ap[0]), [0, NLT], [1, 128]],
                )
                rho_b = bass.AP(
                    tensor=rho_inv.tensor,
                    offset=rho_inv.offset,
                    ap=[list(rho_inv.ap[0]), [1, NLT], [0, 128]],
                )
                nc.gpsimd.tensor_tensor(
                    out=D_all, in0=ident_b, in1=rho_b, op=OP.mult
                )

                def D(i):
                    return D_all[:, 128 * i : 128 * (i + 1)]

                # ---- T~ = Z^T-contraction with Ct_ext; col 256 = kappa free ----
                T_psum = []
                for j in range(NMT):
                    pT = ps_m.tile([128, H + 1], f32, tag="m")
                    for i in range(NLT):
                        mmb(
                            pT,
                            E_lm[i][:, 128 * j : 128 * (j + 1)],
                            CtT(i),
                            (i == 0),
                            (i == NLT - 1),
                        )
                    T_psum.append(pT)
                kap_inv = sb.tile([128, NMT], f32, tag="kapi")
                for j in range(NMT):
                    nc.vector.reciprocal(
                        kap_inv[:, j : j + 1], T_psum[j][:, H : H + 1]
                    )
                T_nat = []
                for j in range(NMT):
                    t = sb.tile([128, H], bf16, tag=f"tn{j}")
                    nc.vector.tensor_scalar_mul(
                        t, T_psum[j][:, 0:H], kap_inv[:, j : j + 1]
                    )
                    T_nat.append(t)

                # ---- P^T packed: P_ml[j][:, 128i+p] = Z[128i+p, 128j+m]*rho_inv
                #      via matmul E_lm[i]^T @ D[i] (scaled transpose) ----
                P_ml = []
                for j in range(NMT):
                    t = sb.tile([128, LC], bf16, tag=f"pml{j}")
                    for c in range(2):
                        pp = ps_g.tile([128, 512], f32, tag="g")
                        for q in range(4):
                            i = 4 * c + q
                            mmb(
                                pp[:, 128 * q : 128 * (q + 1)],
                                E_lm[i][:, 128 * j : 128 * (j + 1)],
                                D(i),
                                True,
                                True,
                            )
                        nc.vector.tensor_copy(
                            out=t[:, 512 * c : 512 * (c + 1)], in_=pp
                        )
                    P_ml.append(t)

                # ---- A^T and Bv^T  [h, l] ----
                A_T = []
                Bv_T = []
                for t_i in range(KH):
                    pA = ps_g.tile([128, 512], f32, tag="g")
                    pA2 = ps_g.tile([128, 512], f32, tag="g")
                    for c, pp in enumerate((pA, pA2)):
                        for j in range(NMT):
                            mmb(
                                pp,
                                QT_sb[:, 256 * j + 128 * t_i : 256 * j + 128 * (t_i + 1)],
                                P_ml[j][:, 512 * c : 512 * (c + 1)],
                                (j == 0),
                                (j == NMT - 1),
                            )
                    a = sb.tile([128, LC], bf16, tag=f"at{t_i}")
                    nc.vector.tensor_copy(out=a[:, 0:512], in_=pA)
                    nc.vector.tensor_copy(out=a[:, 512:1024], in_=pA2)
                    A_T.append(a)
                for t_i in range(KH):
                    pB = ps_g.tile([128, 512], f32, tag="g")
                    pB2 = ps_g.tile([128, 512], f32, tag="g")
                    for c, pp in enumerate((pB, pB2)):
                        for j in range(NMT):
                            mmb(
                                pp,
                                T_nat[j][:, 128 * t_i : 128 * (t_i + 1)],
                                P_ml[j][:, 512 * c : 512 * (c + 1)],
                                (j == 0),
                                (j == NMT - 1),
                            )
                    bv = sb.tile([128, LC], bf16, tag=f"bvt{t_i}")
                    nc.scalar.copy(out=bv[:, 0:512], in_=pB)
                    nc.scalar.copy(out=bv[:, 512:1024], in_=pB2)
                    Bv_T.append(bv)

                # ---- products (DVE bf16, per 512-chunk so final(c=0) starts early) ----
                CA1 = []
                CB1 = []
                for t_i in range(KH):
                    p1 = sb.tile([128, LC], bf16, tag=f"ca1{t_i}")
                    p2 = sb.tile([128, LC], bf16, tag=f"cb1{t_i}")
                    for c in range(2):
                        sl = slice(512 * c, 512 * (c + 1))
                        nc.vector.tensor_mul(p1[:, sl], C_bf[t_i][:, sl], A_T[t_i][:, sl])
                        nc.vector.tensor_mul(p2[:, sl], C_bf[t_i][:, sl], Bv_T[t_i][:, sl])
                    CA1.append(p1)
                    CB1.append(p2)

                # ---- final matmul + relu + store (bf16 out) ----
                blocks = [
                    C_bf[0],
                    C_bf[1],
                    A_T[0],
                    A_T[1],
                    CA1[0],
                    CA1[1],
                    CB1[0],
                    CB1[1],
                ]
                for t_i in range(KH):
                    for c in range(2):
                        po = ps_g.tile([128, 512], f32, tag="g")
                        for f in range(8):
                            mmb(
                                po,
                                WT(f)[:, 128 * t_i : 128 * (t_i + 1)],
                                blocks[f][:, 512 * c : 512 * (c + 1)],
                                (f == 0),
                                (f == 7),
                            )
                        o = sb.tile([128, 512], bf16, tag=f"osb{t_i}{c}")
                        nc.scalar.activation(
                            out=o,
                            in_=po,
                            func=AF.Relu,
                            bias=b_col[:, t_i : t_i + 1],
                        )
                        nc.sync.dma_start(
                            out=out.ap()[
                                b, 128 * t_i : 128 * (t_i + 1), 512 * c : 512 * (c + 1)
                            ],
                            in_=o,
                        )

            def body(iv=None):
                # All input DMAs issue up front (dedicated BB-deep pool) so no
                # load ever queues behind an output store on the DMA track.
                # W_res DMAs slot in right after batch 0's loads.
                st = [load(0)]
                if not _Wn:
                    load_W()
                for b in range(1, BB):
                    st.append(load(b))
                frontend(0, st[0])
                frontend(1, st[1])
                if not WT_sb:
                    build_WT()
                backend(0, st[0])
                frontend(2, st[2])
                backend(1, st[1])
                frontend(3, st[3])
                backend(2, st[2])
                backend(3, st[3])

            if reps == 1:
                body()
            else:
                # PE body is ~1.1k instructions (>4 IRAM blocks): hint the
                # back-edge so each iteration doesn't stall on an I$ refetch.
                with tc.For_i(
                    0,
                    reps,
                    1,
                    hint_engines=(mybir.EngineType.PE,),
                    staggered_reset=True,
                ) as iv:
                    body(iv)

    nc.compile()
    return nc


def _get_nc(reps: int = 1):
    key = ("nc", reps)
    if key not in _CACHE:
        _CACHE[key] = _build(reps)
    return _CACHE[key]


def kernel(C, Q, cmask, qmask, w, W_res, b_res, _reps: int = 1, _want_res: bool = False):
    from concourse.bass_utils import run_bass_kernel_spmd

    nc = _get_nc(_reps)

    C = np.ascontiguousarray(C, dtype=np.float32)
    Q = np.ascontiguousarray(Q, dtype=np.float32)
    w = np.ascontiguousarray(w, dtype=np.float32)
    W_res = np.ascontiguousarray(W_res, dtype=np.float32)
    b_res = np.ascontiguousarray(b_res, dtype=np.float32)

    in_maps = []
    for i in range(N_CORES):
        sl = slice(i * BB, (i + 1) * BB)
        in_maps.append(
            {"C": C[sl], "Q": Q[sl], "w": w, "W_res": W_res, "b_res": b_res}
        )

    res = run_bass_kernel_spmd(nc, in_maps, core_ids=list(range(N_CORES)))
    outs = [
        np.asarray(res.results[i]["out"]).astype(np.float32) for i in range(N_CORES)
    ]
    out = np.concatenate(outs, axis=0)
    if _want_res:
        return out, res
    return out
